# revision 8
# baseline (speedup 1.0000x reference)
"""Trainium2 Bass kernel for nn_DualSampling (topk_masking).

Reference computation (N=8192, D=128, TAU=0.2, K=10):
    proj = user_emb @ W.T + b
    sim  = (proj @ proj.T) / TAU
    sim  = where(mask, -1e9, sim)          # mask = eye + social edges scatter
    y    = softmax((sim + gumbel) / TAU)
    out  = scatter(ones, top_k(y, 10))     # binary [N, N]

Key algebraic reductions used here:
  * softmax and the final /TAU are strictly monotone per row, so the top-10
    index set of y equals the top-10 index set of z = sim + gumbel.
  * The sparse mask is folded into gumbel on the host (subtract 1e9 at
    masked positions) so the device never materializes the mask.
  * The top-10 per row is found exactly via the DVE max8 instruction:
    per 512-wide segment take the top-8 (16 segments), then top-8 of the
    128 candidates, match_replace them to -inf, top-8 again; the 10th
    largest of the row is the 2nd element of the second batch.  This fails
    only if >8 of a row's true top-10 land in one 512-segment
    (P ~ 2e-9 per row for continuous random data).
  * The binary output row is then just (z >= t10), an elementwise compare.

Sharding: rows are split across the 8 NeuronCores (1024 rows each).  Every
core redundantly computes the full projT = W @ user_emb.T + b (cheap) so no
collectives are needed; each core's stripe matmuls read its own row block
as the stationary operand.
"""

import numpy as np

N = 8192
D = 128
TAU = 0.2
NCORES = 8
R = N // NCORES          # rows per core
P = 128                  # partitions / stripe height
CB = 512                 # column block (one PSUM bank of fp32)
BIG = np.float32(1.0e9)
NEG = -3.0e38            # replacement value for extracted maxima
# The reference runs jax on the neuron backend; its fp32 softmax flushes
# exp(d) to zero for d <= -97.28622 (measured boundary: bits c2c2928b -> 0,
# c2c2928a -> positive).  Entries below the cutoff have y == 0 exactly and
# jax.lax.top_k then fills remaining slots with the lowest-index zeros.
# Our z is in (sim + gumbel) scale = d_full / 5.
D_MINE = float(np.float32(-97.286217 / 5.0))   # -19.457243
FILLW = 64               # fill window (first columns); needed fill idx < 19

_cache = {}


def _build(n=N, r=R, g_bufs=6, o_bufs=6, ps_bufs=8, z_bufs=2, cb=CB):
    """Build + compile the single-core SPMD program (all cores identical)."""
    from contextlib import ExitStack

    import concourse.bass as bass
    import concourse.tile as tile
    from concourse import bacc, mybir
    from concourse.alu_op_type import AluOpType

    f32 = mybir.dt.float32
    ncb = n // cb            # column blocks
    spr = r // P             # stripes per core
    ncb_l = r // cb          # column blocks of the local row strip

    nc = bacc.Bacc("TRN2", target_bir_lowering=False, debug=False)
    uT_d = nc.dram_tensor("uT", [D, n], f32, kind="ExternalInput").ap()
    uTl_d = nc.dram_tensor("uTl", [D, r], f32, kind="ExternalInput").ap()
    wT_d = nc.dram_tensor("wT", [D, D], f32, kind="ExternalInput").ap()
    b_d = nc.dram_tensor("b", [D, 1], f32, kind="ExternalInput").ap()
    g_d = nc.dram_tensor("g", [r, n], f32, kind="ExternalInput").ap()
    out_d = nc.dram_tensor("out", [r, n], f32, kind="ExternalOutput").ap()

    with tile.TileContext(nc) as tc, ExitStack() as ctx:
        const = ctx.enter_context(tc.tile_pool(name="const", bufs=1))
        zp = ctx.enter_context(tc.tile_pool(name="zp", bufs=z_bufs))
        vp = ctx.enter_context(tc.tile_pool(name="vp", bufs=2))
        gp = ctx.enter_context(tc.tile_pool(name="gp", bufs=g_bufs))
        op = ctx.enter_context(tc.tile_pool(name="op", bufs=o_bufs))
        pp = ctx.enter_context(tc.tile_pool(name="pp", bufs=ps_bufs, space="PSUM"))

        # ---- prologue: load constants, compute projT (full) and projTl ----
        uT = const.tile([D, n], f32, tag="uT")
        nc.sync.dma_start(uT[:], uT_d)
        uTl = const.tile([D, r], f32, tag="uTl")
        nc.sync.dma_start(uTl[:], uTl_d)
        wT = const.tile([D, D], f32, tag="wT")
        nc.sync.dma_start(wT[:], wT_d)
        bcol = const.tile([D, 1], f32, tag="bcol")
        nc.sync.dma_start(bcol[:], b_d)

        projT = const.tile([D, n], f32, tag="projT")
        projTl = const.tile([D, r], f32, tag="projTl")
        for src, dst, nblk in ((uT, projT, ncb), (uTl, projTl, ncb_l)):
            for c in range(nblk):
                ps = pp.tile([P, cb], f32, tag="ps")
                nc.tensor.matmul(
                    ps[:], wT[:], src[:, c * cb:(c + 1) * cb],
                    start=True, stop=True,
                )
                # projT = W @ uT + b   (PSUM -> SBUF with per-partition bias)
                nc.vector.tensor_scalar_add(dst[:, c * cb:(c + 1) * cb], ps[:], bcol[:, 0:1])

        # ---- main: per 128-row stripe ----
        for s in range(spr):
            lhsT = projTl[:, s * P:(s + 1) * P]
            z = zp.tile([P, n], f32, tag="z")
            V = vp.tile([P, ncb * 8], f32, tag="V")
            for c in range(ncb):
                cs = slice(c * cb, (c + 1) * cb)
                gt = gp.tile([P, cb], f32, tag="gt")
                nc.sync.dma_start(gt[:], g_d[s * P:(s + 1) * P, cs])
                ps = pp.tile([P, cb], f32, tag="ps")
                nc.tensor.matmul(ps[:], lhsT, projT[:, cs], start=True, stop=True)
                # z = S * (1/TAU) + gumbel   (fused, PSUM -> SBUF)
                nc.vector.scalar_tensor_tensor(
                    z[:, cs], ps[:], 1.0 / TAU, gt[:],
                    op0=AluOpType.mult, op1=AluOpType.add,
                )
                nc.vector.max(V[:, c * 8:(c + 1) * 8], z[:, cs])
            # merge candidates: 10th largest value of the row
            m1 = vp.tile([P, 8], f32, tag="m1")
            nc.vector.max(m1[:], V[:])
            V2 = vp.tile([P, ncb * 8], f32, tag="V2")
            nc.vector.match_replace(V2[:], m1[:], V[:], NEG)
            m2 = vp.tile([P, 8], f32, tag="m2")
            nc.vector.max(m2[:], V2[:])
            t10 = m2[:, 1:2]
            # softmax-underflow cutoff C = rowmax + D, threshold T = max(t10, C)
            sc = vp.tile([P, 4], f32, tag="sc")   # [C, T, cnt, f]
            nc.vector.tensor_scalar_add(sc[:, 0:1], m1[:, 0:1], D_MINE)
            nc.vector.tensor_tensor(sc[:, 1:2], m2[:, 1:2], sc[:, 0:1], op=AluOpType.max)
            # cnt = min(#candidates >= C, 10); f = 10 - cnt
            cV = vp.tile([P, ncb * 8], f32, tag="cV")
            nc.vector.tensor_scalar(cV[:], V[:], sc[:, 0:1], None, op0=AluOpType.is_ge)
            nc.vector.tensor_reduce(sc[:, 2:3], cV[:], axis=mybir.AxisListType.X,
                                    op=AluOpType.add)
            # negf = min(cnt, 10) - 10   (= -fill_count)
            nc.vector.tensor_scalar(sc[:, 3:4], sc[:, 2:3], 10.0, 10.0,
                                    op0=AluOpType.min, op1=AluOpType.subtract)
            # fill mask over the first FILLW columns: b & (cumsum(b) <= f)
            fb = vp.tile([P, FILLW], f32, tag="fb")
            nc.vector.tensor_scalar(fb[:], z[:, 0:FILLW], sc[:, 0:1], None,
                                    op0=AluOpType.is_lt)
            fcum = vp.tile([P, FILLW], f32, tag="fcum")
            nc.vector.tensor_tensor_scan(fcum[:], fb[:], fb[:], 0.0,
                                         op0=AluOpType.add, op1=AluOpType.bypass)
            # fsel = (cumsum + negf <= 0) & b
            fsel = vp.tile([P, FILLW], f32, tag="fsel")
            nc.vector.tensor_scalar(fsel[:], fcum[:], sc[:, 3:4], 0.0,
                                    op0=AluOpType.add, op1=AluOpType.is_le)
            nc.vector.tensor_tensor(fsel[:], fsel[:], fb[:], op=AluOpType.mult)
            T = sc[:, 1:2]
            for c in range(ncb):
                cs = slice(c * cb, (c + 1) * cb)
                ot = op.tile([P, cb], f32, tag="ot")
                nc.gpsimd.tensor_scalar(ot[:], z[:, cs], T, None, op0=AluOpType.is_ge)
                if c == 0:
                    nc.vector.tensor_tensor(ot[:, 0:FILLW], ot[:, 0:FILLW], fsel[:],
                                            op=AluOpType.add)
                nc.sync.dma_start(out_d[s * P:(s + 1) * P, cs], ot[:])

    nc.compile()
    return nc


def _host_prep(user_emb, social_edges, W, b, gumbel, n=N, r=R):
    ncores = n // r
    uT = np.ascontiguousarray(user_emb.T).astype(np.float32, copy=False)
    wT = np.ascontiguousarray(W.T).astype(np.float32, copy=False)
    bcol = np.ascontiguousarray(b.astype(np.float32).reshape(D, 1))
    g = np.array(gumbel, dtype=np.float32, copy=True)
    e0 = np.asarray(social_edges[0], dtype=np.int64)
    e1 = np.asarray(social_edges[1], dtype=np.int64)
    g[e0, e1] -= BIG
    idx = np.arange(n)
    g[idx, idx] -= BIG
    in_maps = []
    for i in range(ncores):
        in_maps.append({
            "uT": uT,
            "uTl": np.ascontiguousarray(uT[:, i * r:(i + 1) * r]),
            "wT": wT,
            "b": bcol,
            "g": g[i * r:(i + 1) * r],
        })
    return in_maps


def kernel(user_emb, item_emb, social_edges, W, b, gumbel):
    from concourse import bass_utils

    if "nc" not in _cache:
        _cache["nc"] = _build()
    nc = _cache["nc"]
    in_maps = _host_prep(user_emb, social_edges, W, b, gumbel)
    res = bass_utils.run_bass_kernel_spmd(nc, in_maps, core_ids=list(range(NCORES)))
    out = np.concatenate([res.results[i]["out"] for i in range(NCORES)], axis=0)
    return np.ascontiguousarray(out, dtype=np.float32)
